# revision 38
# baseline (speedup 1.0000x reference)
"""GridPoolingLayer kernel for Trainium2 (8 NeuronCores, Bass/Tile).

Semantics (from the grid-pooling reference): the 1D binary masks partition
H/W into maximal runs of constant value; the layer replaces every grid cell
with its mean (keep_size=True).  The op is separable: out = R @ X @ C per
channel, with R/C block "segment mean broadcast" matrices derived from the
tiny masks, which we compute on the host.

Device strategy per core (channels sharded 8 ways, 32 ch/core).  W is
pre-permuted host-side (within each of NSUPER contiguous "super-blocks")
so col segments of equal length are adjacent; the OUTPUT stays permuted
and the host undoes the permutation after gathering.

  A) row pooling   p1 = P_r @ X        -- PE matmul, P_r^T (with 1/len
     folded in) precomputed host-side, contraction over H on partitions.
  B) col pooling   poolB = segsum_w p1 -- one DVE tensor_reduce per
     length class (classes are contiguous after the permutation).
  C) col expand    cd = poolB[seg]/L   -- one DVE broadcast multiply per
     length class, still in permuted w order.
  D) row expand    y[hchunk] = R_e @ cd -- PE matmul with one-hot R_e^T
     (contraction over row-segment ids), PSUM copied to SBUF on
     scalar/vector alternately, then ONE big DMA per (super, h-chunk)
     issued on the scalar engine's HWDGE queue (separate ring from the
     input loads on sync) so loads and stores overlap.

No collectives: every core runs the same program on its channel slice.
"""

import math
import numpy as np

H, W, C = 512, 512, 256
NCORES = 8
CS = C // NCORES  # 32 channels per core
P = 128

NSUPER = 4       # independent W super-blocks (~128 w's each)
XT_BUFS = 7      # single tag
P1_BUFS = 3      # per-m tag (x2)
PB_BUFS = 2      # per-m tag (x2)
YO_BUFS = 4      # single tag


def _segments(mask):
    m = np.asarray(mask).ravel()
    change = np.nonzero(m[1:] != m[:-1])[0] + 1
    bounds = np.concatenate([[0], change, [len(m)]]).astype(np.int64)
    return [(int(bounds[i]), int(bounds[i + 1])) for i in range(len(bounds) - 1)]


def _plan(row_segs, col_segs):
    """Host-side geometry planning shared by program build + data prep."""
    from collections import defaultdict

    S_h, S_w = len(row_segs), len(col_segs)
    Mh = math.ceil(S_h / P)
    Kh = math.ceil(H / P)

    # ---- split col segs into NSUPER contiguous groups of ~W/NSUPER w's
    supers = []
    target = W / NSUPER
    cur = []
    acc = 0
    for t, (u, v) in enumerate(col_segs):
        cur.append(t)
        acc += v - u
        if acc >= target * (len(supers) + 1) - 1e-9 and len(supers) < NSUPER - 1:
            supers.append(cur)
            cur = []
    supers.append(cur)
    supers = [s for s in supers if s]

    wperm = np.empty(W, dtype=np.int64)
    sb_plans = []
    for ts_all in supers:
        sw0 = col_segs[ts_all[0]][0]          # super start (original w)
        swid = col_segs[ts_all[-1]][1] - sw0  # super width

        by_len = defaultdict(list)
        for t in ts_all:
            u, v = col_segs[t]
            by_len[v - u].append(t)
        # class-sorted within the super: segs of equal length adjacent
        off = sw0
        runs = []   # (L, n, lw0, slot0): n segs of length L at permuted
        slot0 = 0   # local w offset lw0, poolB slot slot0
        for L in sorted(by_len):
            ts = by_len[L]
            runs.append((L, len(ts), off - sw0, slot0))
            for t in ts:
                u, v = col_segs[t]
                wperm[off:off + (v - u)] = np.arange(u, v)
                off += v - u
            slot0 += len(ts)
        sb_plans.append(dict(n_segs=len(ts_all), sw0=sw0, swid=swid,
                             runs=runs))

    # ---- which h-chunks feed each s-chunk (phase A contraction)
    overlap = []
    for m in range(Mh):
        s_lo = m * P
        s_hi = min(S_h, (m + 1) * P)
        h_lo = row_segs[s_lo][0]
        h_hi = row_segs[s_hi - 1][1]
        ks = [k for k in range(Kh) if k * P < h_hi and (k + 1) * P > h_lo]
        overlap.append(ks)

    # ---- which s-chunks feed each h-chunk (phase D contraction)
    overlap_inv = []
    for k in range(Kh):
        ms = sorted({
            s // P
            for s, (a, b) in enumerate(row_segs)
            if a < (k + 1) * P and b > k * P
        })
        overlap_inv.append(ms)

    return dict(
        S_h=S_h, S_w=S_w, Mh=Mh, Kh=Kh,
        supers=sb_plans, overlap=overlap, overlap_inv=overlap_inv,
        wperm=wperm,
    )


def _build_program(row_segs, col_segs, plan):
    import concourse.bass as bass
    import concourse.mybir as mybir
    import concourse.tile as tile

    fp32 = mybir.dt.float32
    bf16 = mybir.dt.bfloat16
    COPY = mybir.ActivationFunctionType.Copy
    ADD = mybir.AluOpType.add
    AXX = mybir.AxisListType.X

    Mh, Kh = plan["Mh"], plan["Kh"]
    FW = W * CS  # full row free size (16384)

    from concourse import bacc

    nc = bacc.Bacc()
    x = nc.dram_tensor("x", [H, FW], bf16, kind="ExternalInput")
    prT = nc.dram_tensor("prT", [H, Mh * P], bf16, kind="ExternalInput")
    reT = nc.dram_tensor("reT", [Mh * P, H], bf16, kind="ExternalInput")
    y = nc.dram_tensor("y", [H, FW], bf16, kind="ExternalOutput")

    sp0 = plan["supers"][0]
    with tile.TileContext(nc) as tc:
        with (
            tc.tile_pool(name="consts", bufs=1) as consts,
            tc.tile_pool(name="xin", bufs=XT_BUFS) as xin,
            tc.tile_pool(name="p1", bufs=P1_BUFS) as p1pool,
            tc.tile_pool(name="pB", bufs=PB_BUFS) as pBpool,
            tc.tile_pool(name="pBs", bufs=PB_BUFS) as pBspool,
            tc.tile_pool(name="yo", bufs=YO_BUFS) as yopool,
            tc.tile_pool(name="ps", bufs=7, space="PSUM") as pspool,
            tc.tile_pool(name="warm", bufs=1, space="PSUM") as warmpool,
        ):
            # first super's input loads go FIRST on the sync queue so
            # the big x stream starts before the (tiny) const loads
            xts0 = []
            fw0 = sp0["swid"] * CS
            c00 = sp0["sw0"] * CS
            for k in range(Kh):
                xt = xin.tile([P, fw0], bf16, tag="xt", name=f"xt0_{k}")
                nc.sync.dma_start(xt[:], x[k * P:(k + 1) * P, c00:c00 + fw0])
                xts0.append(xt)

            # stationary pooling/expansion matrices
            prT_sb = []
            for k in range(Kh):
                t = consts.tile([P, Mh * P], bf16, name=f"prT{k}")
                nc.sync.dma_start(t[:], prT[k * P:(k + 1) * P, :])
                prT_sb.append(t)
            reT_sb = []
            for m in range(Mh):
                t = consts.tile([P, H], bf16, name=f"reT{m}")
                nc.sync.dma_start(t[:], reT[m * P:(m + 1) * P, :])
                reT_sb.append(t)

            # PE pre-touch of every DMA-produced matmul operand: later
            # matmuls then reach it without a DMA sync-wait (keeps the
            # LDWEIGHTS wait count within the ISA limit).
            ps_warm = warmpool.tile([1, 512], fp32, name="ps_warm")

            def warm(t):
                nc.tensor.matmul(
                    ps_warm[:1, :1], t[:, :1], t[:, :1], start=True, stop=True
                )

            for t in prT_sb + reT_sb + xts0:
                warm(t)

            ncopy = 0
            for si, sp in enumerate(plan["supers"]):
                fw = sp["swid"] * CS
                c0 = sp["sw0"] * CS

                # ---- loads (sync queue), one [128, fw] tile per h-chunk
                if si == 0:
                    xts = xts0
                else:
                    xts = []
                    for k in range(Kh):
                        xt = xin.tile([P, fw], bf16, tag="xt",
                                      name=f"xt{si}_{k}")
                        nc.sync.dma_start(xt[:],
                                          x[k * P:(k + 1) * P, c0:c0 + fw])
                        warm(xt)
                        xts.append(xt)

                # ---- A: row pooling into s-partition space
                p1s = []
                for m in range(Mh):
                    p1 = p1pool.tile([P, fw], bf16, tag=f"p1_{m % 2}",
                                     name=f"p1_{si}_{m}")
                    ks = plan["overlap"][m]
                    for n0 in range(0, fw, 512):
                        nw = min(512, fw - n0)
                        ps = pspool.tile([P, 512], fp32, tag="ps",
                                         name=f"psA{si}_{m}_{n0}")
                        for i, k in enumerate(ks):
                            nc.tensor.matmul(
                                ps[:, :nw],
                                prT_sb[k][:, m * P:(m + 1) * P],
                                xts[k][:, n0:n0 + nw],
                                start=(i == 0),
                                stop=(i == len(ks) - 1),
                            )
                        nc.scalar.activation(p1[:, n0:n0 + nw], ps[:, :nw],
                                             COPY)
                    p1s.append(p1)

                # ---- B: col pooling (one reduce per length class)
                pBs = []
                for m in range(Mh):
                    pB = pBpool.tile([P, sp["n_segs"] * CS], fp32,
                                     tag=f"pB{m % 2}", name=f"pB{si}_{m}")
                    for (L, n, lw0, slot0) in sp["runs"]:
                        if L == 1:
                            continue  # pooled value == p1 column
                        src = p1s[m][:, lw0 * CS:(lw0 + n * L) * CS]
                        src = src.rearrange("p (j l c) -> p j c l",
                                            j=n, l=L, c=CS)
                        dst = pB[:, slot0 * CS:(slot0 + n) * CS]
                        dst = dst.rearrange("p (j c) -> p j c", j=n, c=CS)
                        nc.vector.tensor_reduce(dst, src, axis=AXX, op=ADD)
                    pBs.append(pB)

                # ---- Bs: scale pooled cols by 1/L and cast to bf16
                pBss = []
                for m in range(Mh):
                    pbs = pBspool.tile([P, sp["n_segs"] * CS], bf16,
                                       tag=f"pBs{m % 2}", name=f"pBs{si}_{m}")
                    for (L, n, lw0, slot0) in sp["runs"]:
                        sl = slice(slot0 * CS, (slot0 + n) * CS)
                        if L == 1:
                            nc.scalar.activation(
                                pbs[:, sl],
                                p1s[m][:, lw0 * CS:(lw0 + n) * CS], COPY)
                        else:
                            nc.scalar.activation(pbs[:, sl], pBs[m][:, sl],
                                                 COPY, scale=1.0 / L)
                    pBss.append(pbs)

                # ---- D: row expand + big stores.  The col expansion is
                # folded into the matmul's moving operand: the rhs reads
                # pBs with a step-0 broadcast AP per class-run piece, so
                # the full [h, w, c] cell value streams out of PE directly.
                pieces = []   # (out_w0, jj, L, slot_j0), contiguous in w
                for (L, n, lw0, slot0) in sp["runs"]:
                    jj_max = max(1, 512 // (L * CS))
                    j = 0
                    while j < n:
                        jj = min(jj_max, n - j)
                        pieces.append((lw0 + j * L, jj, L, slot0 + j))
                        j += jj
                # pack contiguous pieces into <=512-elem PSUM slots
                slots = []
                cur = []
                acc = 0
                for pc in pieces:
                    wid = pc[1] * pc[2] * CS
                    if acc + wid > 512:
                        slots.append((cur, acc))
                        cur, acc = [], 0
                    cur.append(pc)
                    acc += wid
                if cur:
                    slots.append((cur, acc))

                for k in range(Kh):
                    ms = plan["overlap_inv"][k]
                    yo = yopool.tile([P, fw], bf16, tag="yo",
                                     name=f"yo{si}_{k}")
                    for (slot_pcs, used) in slots:
                        out0 = slot_pcs[0][0] * CS
                        ps = pspool.tile([P, 512], fp32, tag="ps",
                                         name=f"psD{si}_{k}_{out0}")
                        po = 0
                        for (w0, jj, L, sj0) in slot_pcs:
                            wid = jj * L * CS
                            for i, m in enumerate(ms):
                                src = pBss[m][:, sj0 * CS:(sj0 + jj) * CS]
                                src = src.rearrange("p (j c) -> p j c",
                                                    j=jj, c=CS)
                                src = src.unsqueeze(2).broadcast_to(
                                    [P, jj, L, CS])
                                nc.tensor.matmul(
                                    ps[:, po:po + wid],
                                    reT_sb[m][:, k * P:(k + 1) * P],
                                    src,
                                    start=(i == 0),
                                    stop=(i == len(ms) - 1),
                                )
                            po += wid
                        if ncopy % 2 == 0:
                            nc.scalar.activation(yo[:, out0:out0 + used],
                                                 ps[:, :used], COPY)
                        else:
                            nc.vector.tensor_scalar_mul(
                                yo[:, out0:out0 + used], ps[:, :used], 1.0)
                        ncopy += 1
                    # stores on SWDGE (gpsimd) -- own DMA queue, and the
                    # otherwise-idle gpsimd NX absorbs the data-ready
                    # waits instead of convoying the scalar queue
                    if si == len(plan["supers"]) - 1 and k == Kh - 1:
                        hh = (fw // (2 * CS)) * CS
                        nc.gpsimd.dma_start(
                            y[k * P:(k + 1) * P, c0:c0 + hh], yo[:, :hh])
                        nc.gpsimd.dma_start(
                            y[k * P:(k + 1) * P, c0 + hh:c0 + fw],
                            yo[:, hh:])
                    else:
                        nc.gpsimd.dma_start(
                            y[k * P:(k + 1) * P, c0:c0 + fw], yo[:])

    nc.compile()
    nc.finalize()
    return nc


def _host_arrays(input, plan, row_segs):
    import ml_dtypes

    Mh = plan["Mh"]
    prT = np.zeros((H, Mh * P), dtype=np.float32)
    reT = np.zeros((Mh * P, H), dtype=np.float32)
    for s, (a, b) in enumerate(row_segs):
        prT[a:b, s] = 1.0 / (b - a)
        reT[s, a:b] = 1.0
    prT = prT.astype(ml_dtypes.bfloat16)
    reT = reT.astype(ml_dtypes.bfloat16)
    # host W permutation (class-sorted within supers), per-core ch slices
    xp = np.asarray(input)[0][:, plan["wperm"], :].astype(ml_dtypes.bfloat16)
    in_maps = []
    for k in range(NCORES):
        xc = np.ascontiguousarray(xp[:, :, k * CS:(k + 1) * CS])
        in_maps.append({"x": xc.reshape(H, W * CS), "prT": prT, "reT": reT})
    return in_maps


def _prep_host(input, h_mask, v_mask):
    """Returns (nc, in_maps, plan) ready for execution."""
    row_segs = _segments(h_mask)
    col_segs = _segments(v_mask)
    plan = _plan(row_segs, col_segs)

    in_maps = _host_arrays(input, plan, row_segs)
    nc = _build_program(row_segs, col_segs, plan)
    return nc, in_maps, plan


# stash for test.py introspection
LAST_RESULT = {}
LAST_NC = {}
_EXEC_CACHE = {}


def _make_executable(nc):
    """Build a reusable sharded jit callable for this program.

    Mirrors bass2jax.run_bass_via_pjrt's multi-core branch but keeps the
    jitted function so repeated calls skip retrace/recompile (and so the
    test harness can time steady-state executions).
    """
    import jax
    import concourse.mybir as mybir
    from concourse import bass2jax
    from jax.sharding import Mesh, PartitionSpec
    from jax.experimental.shard_map import shard_map

    bass2jax.install_neuronx_cc_hook()

    partition_name = (
        nc.partition_id_tensor.name if nc.partition_id_tensor else None
    )
    in_names, out_names, out_shapes, out_dtypes = [], [], [], []
    for alloc in nc.m.functions[0].allocations:
        if not isinstance(alloc, mybir.MemoryLocationSet):
            continue
        name = alloc.memorylocations[0].name
        if alloc.kind == "ExternalInput":
            if name != partition_name:
                in_names.append(name)
        elif alloc.kind == "ExternalOutput":
            out_names.append(name)
            out_shapes.append(tuple(alloc.tensor_shape))
            out_dtypes.append(mybir.dt.np(alloc.dtype))
    out_avals = tuple(
        jax.core.ShapedArray(s, d) for s, d in zip(out_shapes, out_dtypes)
    )
    n_params = len(in_names)
    n_outs = len(out_names)
    all_names = in_names + out_names
    if partition_name is not None:
        all_names = all_names + [partition_name]

    def _body(*args):
        operands = list(args)
        if partition_name is not None:
            operands.append(bass2jax.partition_id_tensor())
        outs = bass2jax._bass_exec_p.bind(
            *operands,
            out_avals=out_avals,
            in_names=tuple(all_names),
            out_names=tuple(out_names),
            lowering_input_output_aliases=(),
            sim_require_finite=True,
            sim_require_nnan=True,
            nc=nc,
        )
        return tuple(outs)

    devices = jax.devices()[:NCORES]
    mesh = Mesh(np.asarray(devices), ("core",))
    donate = tuple(range(n_params, n_params + n_outs))
    sharded = jax.jit(
        shard_map(
            _body,
            mesh=mesh,
            in_specs=(PartitionSpec("core"),) * (n_params + n_outs),
            out_specs=(PartitionSpec("core"),) * n_outs,
            check_rep=False,
        ),
        donate_argnums=donate,
        keep_unused=True,
    )

    def run(in_maps):
        concat_in = [
            np.concatenate([m[name] for m in in_maps], axis=0)
            for name in in_names
        ]
        concat_zeros = [
            np.zeros((NCORES * s[0], *s[1:]), d)
            for s, d in zip(out_shapes, out_dtypes)
        ]
        out_arrs = sharded(*concat_in, *concat_zeros)
        return [
            {
                name: np.asarray(out_arrs[i]).reshape(
                    NCORES, *out_shapes[i]
                )[c]
                for i, name in enumerate(out_names)
            }
            for c in range(NCORES)
        ]

    return run


def _get_run(input, h_mask, v_mask):
    key = (np.asarray(h_mask).tobytes(), np.asarray(v_mask).tobytes())
    if key not in _EXEC_CACHE:
        nc, in_maps, plan = _prep_host(
            np.asarray(input), np.asarray(h_mask), np.asarray(v_mask)
        )
        LAST_NC["nc"] = nc
        _EXEC_CACHE[key] = (_make_executable(nc), plan)
    else:
        # still need per-call input prep (data may differ between calls)
        row_segs = _segments(h_mask)
        plan = _EXEC_CACHE[key][1]
        in_maps = _host_arrays(input, plan, row_segs)
    return _EXEC_CACHE[key][0], in_maps


def kernel(input, h_mask, v_mask):
    run, in_maps = _get_run(input, h_mask, v_mask)
    results = run(in_maps)
    LAST_RESULT["results"] = results
    plan = _EXEC_CACHE[
        (np.asarray(h_mask).tobytes(), np.asarray(v_mask).tobytes())
    ][1]
    yp = np.concatenate(
        [np.asarray(results[k]["y"]).astype(np.float32).reshape(H, W, CS)
         for k in range(NCORES)],
        axis=-1,
    )
    # undo the host-side W permutation (output is in permuted w order)
    out = np.empty_like(yp)
    out[:, plan["wperm"], :] = yp
    return out[None]


# revision 39
# speedup vs baseline: 1.0244x; 1.0244x over previous
"""GridPoolingLayer kernel for Trainium2 (8 NeuronCores, Bass/Tile).

Semantics (from the grid-pooling reference): the 1D binary masks partition
H/W into maximal runs of constant value; the layer replaces every grid cell
with its mean (keep_size=True).  The op is separable: out = R @ X @ C per
channel, with R/C block "segment mean broadcast" matrices derived from the
tiny masks, which we compute on the host.

Device strategy per core (channels sharded 8 ways, 32 ch/core).  W is
pre-permuted host-side (within each of NSUPER contiguous "super-blocks")
so col segments of equal length are adjacent; the OUTPUT stays permuted
and the host undoes the permutation after gathering.

  A) row pooling   p1 = P_r @ X        -- PE matmul, P_r^T (with 1/len
     folded in) precomputed host-side, contraction over H on partitions.
  B) col pooling   poolB = segsum_w p1 -- one DVE tensor_reduce per
     length class (classes are contiguous after the permutation).
  C) col expand    cd = poolB[seg]/L   -- one DVE broadcast multiply per
     length class, still in permuted w order.
  D) row expand    y[hchunk] = R_e @ cd -- PE matmul with one-hot R_e^T
     (contraction over row-segment ids), PSUM copied to SBUF on
     scalar/vector alternately, then ONE big DMA per (super, h-chunk)
     issued on the scalar engine's HWDGE queue (separate ring from the
     input loads on sync) so loads and stores overlap.

No collectives: every core runs the same program on its channel slice.
"""

import math
import numpy as np

H, W, C = 512, 512, 256
NCORES = 8
CS = C // NCORES  # 32 channels per core
P = 128

NSUPER = 4       # independent W super-blocks (~128 w's each)
XT_BUFS = 7      # single tag
P1_BUFS = 3      # per-m tag (x2)
PB_BUFS = 2      # per-m tag (x2)
YO_BUFS = 4      # single tag


def _segments(mask):
    m = np.asarray(mask).ravel()
    change = np.nonzero(m[1:] != m[:-1])[0] + 1
    bounds = np.concatenate([[0], change, [len(m)]]).astype(np.int64)
    return [(int(bounds[i]), int(bounds[i + 1])) for i in range(len(bounds) - 1)]


def _plan(row_segs, col_segs):
    """Host-side geometry planning shared by program build + data prep."""
    from collections import defaultdict

    S_h, S_w = len(row_segs), len(col_segs)
    Mh = math.ceil(S_h / P)
    Kh = math.ceil(H / P)

    # ---- split col segs into NSUPER contiguous groups of ~W/NSUPER w's
    supers = []
    target = W / NSUPER
    cur = []
    acc = 0
    for t, (u, v) in enumerate(col_segs):
        cur.append(t)
        acc += v - u
        if acc >= target * (len(supers) + 1) - 1e-9 and len(supers) < NSUPER - 1:
            supers.append(cur)
            cur = []
    supers.append(cur)
    supers = [s for s in supers if s]

    wperm = np.empty(W, dtype=np.int64)
    sb_plans = []
    for ts_all in supers:
        sw0 = col_segs[ts_all[0]][0]          # super start (original w)
        swid = col_segs[ts_all[-1]][1] - sw0  # super width

        by_len = defaultdict(list)
        for t in ts_all:
            u, v = col_segs[t]
            by_len[v - u].append(t)
        # class-sorted within the super: segs of equal length adjacent
        off = sw0
        runs = []   # (L, n, lw0, slot0): n segs of length L at permuted
        slot0 = 0   # local w offset lw0, poolB slot slot0
        for L in sorted(by_len):
            ts = by_len[L]
            runs.append((L, len(ts), off - sw0, slot0))
            for t in ts:
                u, v = col_segs[t]
                wperm[off:off + (v - u)] = np.arange(u, v)
                off += v - u
            slot0 += len(ts)
        sb_plans.append(dict(n_segs=len(ts_all), sw0=sw0, swid=swid,
                             runs=runs))

    # ---- which h-chunks feed each s-chunk (phase A contraction)
    overlap = []
    for m in range(Mh):
        s_lo = m * P
        s_hi = min(S_h, (m + 1) * P)
        h_lo = row_segs[s_lo][0]
        h_hi = row_segs[s_hi - 1][1]
        ks = [k for k in range(Kh) if k * P < h_hi and (k + 1) * P > h_lo]
        overlap.append(ks)

    # ---- which s-chunks feed each h-chunk (phase D contraction)
    overlap_inv = []
    for k in range(Kh):
        ms = sorted({
            s // P
            for s, (a, b) in enumerate(row_segs)
            if a < (k + 1) * P and b > k * P
        })
        overlap_inv.append(ms)

    return dict(
        S_h=S_h, S_w=S_w, Mh=Mh, Kh=Kh,
        supers=sb_plans, overlap=overlap, overlap_inv=overlap_inv,
        wperm=wperm,
    )


def _build_program(row_segs, col_segs, plan):
    import concourse.bass as bass
    import concourse.mybir as mybir
    import concourse.tile as tile

    fp32 = mybir.dt.float32
    bf16 = mybir.dt.bfloat16
    COPY = mybir.ActivationFunctionType.Copy
    ADD = mybir.AluOpType.add
    AXX = mybir.AxisListType.X

    Mh, Kh = plan["Mh"], plan["Kh"]
    FW = W * CS  # full row free size (16384)

    from concourse import bacc

    nc = bacc.Bacc()
    x = nc.dram_tensor("x", [H, FW], bf16, kind="ExternalInput")
    prT = nc.dram_tensor("prT", [H, Mh * P], bf16, kind="ExternalInput")
    reT = nc.dram_tensor("reT", [Mh * P, H], bf16, kind="ExternalInput")
    y = nc.dram_tensor("y", [H, FW], bf16, kind="ExternalOutput")

    with tile.TileContext(nc) as tc:
        with (
            tc.tile_pool(name="consts", bufs=1) as consts,
            tc.tile_pool(name="xin", bufs=XT_BUFS) as xin,
            tc.tile_pool(name="p1", bufs=P1_BUFS) as p1pool,
            tc.tile_pool(name="pB", bufs=PB_BUFS) as pBpool,
            tc.tile_pool(name="pBs", bufs=PB_BUFS) as pBspool,
            tc.tile_pool(name="yo", bufs=YO_BUFS) as yopool,
            tc.tile_pool(name="ps", bufs=7, space="PSUM") as pspool,
            tc.tile_pool(name="warm", bufs=1, space="PSUM") as warmpool,
        ):
            # stationary pooling/expansion matrices
            prT_sb = []
            for k in range(Kh):
                t = consts.tile([P, Mh * P], bf16, name=f"prT{k}")
                nc.sync.dma_start(t[:], prT[k * P:(k + 1) * P, :])
                prT_sb.append(t)
            reT_sb = []
            for m in range(Mh):
                t = consts.tile([P, H], bf16, name=f"reT{m}")
                nc.sync.dma_start(t[:], reT[m * P:(m + 1) * P, :])
                reT_sb.append(t)

            # PE pre-touch of every DMA-produced matmul operand: later
            # matmuls then reach it without a DMA sync-wait (keeps the
            # LDWEIGHTS wait count within the ISA limit).
            ps_warm = warmpool.tile([1, 512], fp32, name="ps_warm")

            def warm(t):
                nc.tensor.matmul(
                    ps_warm[:1, :1], t[:, :1], t[:, :1], start=True, stop=True
                )

            for t in prT_sb + reT_sb:
                warm(t)

            ncopy = 0
            for si, sp in enumerate(plan["supers"]):
                fw = sp["swid"] * CS
                c0 = sp["sw0"] * CS

                # ---- loads (sync queue), one [128, fw] tile per h-chunk
                xts = []
                for k in range(Kh):
                    xt = xin.tile([P, fw], bf16, tag="xt", name=f"xt{si}_{k}")
                    nc.sync.dma_start(xt[:], x[k * P:(k + 1) * P, c0:c0 + fw])
                    warm(xt)
                    xts.append(xt)

                # ---- A: row pooling into s-partition space
                p1s = []
                for m in range(Mh):
                    p1 = p1pool.tile([P, fw], bf16, tag=f"p1_{m % 2}",
                                     name=f"p1_{si}_{m}")
                    ks = plan["overlap"][m]
                    for n0 in range(0, fw, 512):
                        nw = min(512, fw - n0)
                        ps = pspool.tile([P, 512], fp32, tag="ps",
                                         name=f"psA{si}_{m}_{n0}")
                        for i, k in enumerate(ks):
                            nc.tensor.matmul(
                                ps[:, :nw],
                                prT_sb[k][:, m * P:(m + 1) * P],
                                xts[k][:, n0:n0 + nw],
                                start=(i == 0),
                                stop=(i == len(ks) - 1),
                            )
                        nc.scalar.activation(p1[:, n0:n0 + nw], ps[:, :nw],
                                             COPY)
                    p1s.append(p1)

                # ---- B: col pooling (one reduce per length class)
                pBs = []
                for m in range(Mh):
                    pB = pBpool.tile([P, sp["n_segs"] * CS], fp32,
                                     tag=f"pB{m % 2}", name=f"pB{si}_{m}")
                    for (L, n, lw0, slot0) in sp["runs"]:
                        if L == 1:
                            continue  # pooled value == p1 column
                        src = p1s[m][:, lw0 * CS:(lw0 + n * L) * CS]
                        src = src.rearrange("p (j l c) -> p j c l",
                                            j=n, l=L, c=CS)
                        dst = pB[:, slot0 * CS:(slot0 + n) * CS]
                        dst = dst.rearrange("p (j c) -> p j c", j=n, c=CS)
                        nc.vector.tensor_reduce(dst, src, axis=AXX, op=ADD)
                    pBs.append(pB)

                # ---- Bs: scale pooled cols by 1/L and cast to bf16
                pBss = []
                for m in range(Mh):
                    pbs = pBspool.tile([P, sp["n_segs"] * CS], bf16,
                                       tag=f"pBs{m % 2}", name=f"pBs{si}_{m}")
                    for (L, n, lw0, slot0) in sp["runs"]:
                        sl = slice(slot0 * CS, (slot0 + n) * CS)
                        if L == 1:
                            nc.scalar.activation(
                                pbs[:, sl],
                                p1s[m][:, lw0 * CS:(lw0 + n) * CS], COPY)
                        else:
                            nc.scalar.activation(pbs[:, sl], pBs[m][:, sl],
                                                 COPY, scale=1.0 / L)
                    pBss.append(pbs)

                # ---- D: row expand + big stores.  The col expansion is
                # folded into the matmul's moving operand: the rhs reads
                # pBs with a step-0 broadcast AP per class-run piece, so
                # the full [h, w, c] cell value streams out of PE directly.
                pieces = []   # (out_w0, jj, L, slot_j0), contiguous in w
                for (L, n, lw0, slot0) in sp["runs"]:
                    jj_max = max(1, 512 // (L * CS))
                    j = 0
                    while j < n:
                        jj = min(jj_max, n - j)
                        pieces.append((lw0 + j * L, jj, L, slot0 + j))
                        j += jj
                # pack contiguous pieces into <=512-elem PSUM slots
                slots = []
                cur = []
                acc = 0
                for pc in pieces:
                    wid = pc[1] * pc[2] * CS
                    if acc + wid > 512:
                        slots.append((cur, acc))
                        cur, acc = [], 0
                    cur.append(pc)
                    acc += wid
                if cur:
                    slots.append((cur, acc))

                for k in range(Kh):
                    ms = plan["overlap_inv"][k]
                    yo = yopool.tile([P, fw], bf16, tag="yo",
                                     name=f"yo{si}_{k}")
                    for (slot_pcs, used) in slots:
                        out0 = slot_pcs[0][0] * CS
                        ps = pspool.tile([P, 512], fp32, tag="ps",
                                         name=f"psD{si}_{k}_{out0}")
                        po = 0
                        for (w0, jj, L, sj0) in slot_pcs:
                            wid = jj * L * CS
                            for i, m in enumerate(ms):
                                src = pBss[m][:, sj0 * CS:(sj0 + jj) * CS]
                                src = src.rearrange("p (j c) -> p j c",
                                                    j=jj, c=CS)
                                src = src.unsqueeze(2).broadcast_to(
                                    [P, jj, L, CS])
                                nc.tensor.matmul(
                                    ps[:, po:po + wid],
                                    reT_sb[m][:, k * P:(k + 1) * P],
                                    src,
                                    start=(i == 0),
                                    stop=(i == len(ms) - 1),
                                )
                            po += wid
                        if ncopy % 2 == 0:
                            nc.scalar.activation(yo[:, out0:out0 + used],
                                                 ps[:, :used], COPY)
                        else:
                            nc.vector.tensor_scalar_mul(
                                yo[:, out0:out0 + used], ps[:, :used], 1.0)
                        ncopy += 1
                    # stores on SWDGE (gpsimd) -- own DMA queue, and the
                    # otherwise-idle gpsimd NX absorbs the data-ready
                    # waits instead of convoying the scalar queue
                    nc.gpsimd.dma_start(y[k * P:(k + 1) * P, c0:c0 + fw],
                                        yo[:])

    nc.compile()
    nc.finalize()
    return nc


def _host_arrays(input, plan, row_segs):
    import ml_dtypes

    Mh = plan["Mh"]
    prT = np.zeros((H, Mh * P), dtype=np.float32)
    reT = np.zeros((Mh * P, H), dtype=np.float32)
    for s, (a, b) in enumerate(row_segs):
        prT[a:b, s] = 1.0 / (b - a)
        reT[s, a:b] = 1.0
    prT = prT.astype(ml_dtypes.bfloat16)
    reT = reT.astype(ml_dtypes.bfloat16)
    # host W permutation (class-sorted within supers), per-core ch slices
    xp = np.asarray(input)[0][:, plan["wperm"], :].astype(ml_dtypes.bfloat16)
    in_maps = []
    for k in range(NCORES):
        xc = np.ascontiguousarray(xp[:, :, k * CS:(k + 1) * CS])
        in_maps.append({"x": xc.reshape(H, W * CS), "prT": prT, "reT": reT})
    return in_maps


def _prep_host(input, h_mask, v_mask):
    """Returns (nc, in_maps, plan) ready for execution."""
    row_segs = _segments(h_mask)
    col_segs = _segments(v_mask)
    plan = _plan(row_segs, col_segs)

    in_maps = _host_arrays(input, plan, row_segs)
    nc = _build_program(row_segs, col_segs, plan)
    return nc, in_maps, plan


# stash for test.py introspection
LAST_RESULT = {}
LAST_NC = {}
_EXEC_CACHE = {}


def _make_executable(nc):
    """Build a reusable sharded jit callable for this program.

    Mirrors bass2jax.run_bass_via_pjrt's multi-core branch but keeps the
    jitted function so repeated calls skip retrace/recompile (and so the
    test harness can time steady-state executions).
    """
    import jax
    import concourse.mybir as mybir
    from concourse import bass2jax
    from jax.sharding import Mesh, PartitionSpec
    from jax.experimental.shard_map import shard_map

    bass2jax.install_neuronx_cc_hook()

    partition_name = (
        nc.partition_id_tensor.name if nc.partition_id_tensor else None
    )
    in_names, out_names, out_shapes, out_dtypes = [], [], [], []
    for alloc in nc.m.functions[0].allocations:
        if not isinstance(alloc, mybir.MemoryLocationSet):
            continue
        name = alloc.memorylocations[0].name
        if alloc.kind == "ExternalInput":
            if name != partition_name:
                in_names.append(name)
        elif alloc.kind == "ExternalOutput":
            out_names.append(name)
            out_shapes.append(tuple(alloc.tensor_shape))
            out_dtypes.append(mybir.dt.np(alloc.dtype))
    out_avals = tuple(
        jax.core.ShapedArray(s, d) for s, d in zip(out_shapes, out_dtypes)
    )
    n_params = len(in_names)
    n_outs = len(out_names)
    all_names = in_names + out_names
    if partition_name is not None:
        all_names = all_names + [partition_name]

    def _body(*args):
        operands = list(args)
        if partition_name is not None:
            operands.append(bass2jax.partition_id_tensor())
        outs = bass2jax._bass_exec_p.bind(
            *operands,
            out_avals=out_avals,
            in_names=tuple(all_names),
            out_names=tuple(out_names),
            lowering_input_output_aliases=(),
            sim_require_finite=True,
            sim_require_nnan=True,
            nc=nc,
        )
        return tuple(outs)

    devices = jax.devices()[:NCORES]
    mesh = Mesh(np.asarray(devices), ("core",))
    donate = tuple(range(n_params, n_params + n_outs))
    sharded = jax.jit(
        shard_map(
            _body,
            mesh=mesh,
            in_specs=(PartitionSpec("core"),) * (n_params + n_outs),
            out_specs=(PartitionSpec("core"),) * n_outs,
            check_rep=False,
        ),
        donate_argnums=donate,
        keep_unused=True,
    )

    def run(in_maps):
        concat_in = [
            np.concatenate([m[name] for m in in_maps], axis=0)
            for name in in_names
        ]
        concat_zeros = [
            np.zeros((NCORES * s[0], *s[1:]), d)
            for s, d in zip(out_shapes, out_dtypes)
        ]
        out_arrs = sharded(*concat_in, *concat_zeros)
        return [
            {
                name: np.asarray(out_arrs[i]).reshape(
                    NCORES, *out_shapes[i]
                )[c]
                for i, name in enumerate(out_names)
            }
            for c in range(NCORES)
        ]

    return run


def _get_run(input, h_mask, v_mask):
    key = (np.asarray(h_mask).tobytes(), np.asarray(v_mask).tobytes())
    if key not in _EXEC_CACHE:
        nc, in_maps, plan = _prep_host(
            np.asarray(input), np.asarray(h_mask), np.asarray(v_mask)
        )
        LAST_NC["nc"] = nc
        _EXEC_CACHE[key] = (_make_executable(nc), plan)
    else:
        # still need per-call input prep (data may differ between calls)
        row_segs = _segments(h_mask)
        plan = _EXEC_CACHE[key][1]
        in_maps = _host_arrays(input, plan, row_segs)
    return _EXEC_CACHE[key][0], in_maps


def kernel(input, h_mask, v_mask):
    run, in_maps = _get_run(input, h_mask, v_mask)
    results = run(in_maps)
    LAST_RESULT["results"] = results
    plan = _EXEC_CACHE[
        (np.asarray(h_mask).tobytes(), np.asarray(v_mask).tobytes())
    ][1]
    yp = np.concatenate(
        [np.asarray(results[k]["y"]).astype(np.float32).reshape(H, W, CS)
         for k in range(NCORES)],
        axis=-1,
    )
    # undo the host-side W permutation (output is in permuted w order)
    out = np.empty_like(yp)
    out[:, plan["wperm"], :] = yp
    return out[None]
